# revision 15
# baseline (speedup 1.0000x reference)
"""Trainium2 Bass kernel for nn_LowRankLayer_dilation (B=4, C=64, H=W=128).

Math (rank-3 NMF collapses exactly; eps negligible):
    h   = relu(W_head @ x)            per-pixel channel matmul
    g   = W_tail @ h
    a   = box9(h)                     3x3 dilation-2 box sum, edge-clamped
    n_k = sum_c (a/9)_c * h_c(p+d_k)  9 taps, d in {-2,0,2}^2
    out = x + (n_4 / sum_j n_j^2) * sum_k n_k * g(p+d_k)

Sharding: pure data parallel, 8 cores = (batch, H-half); 68-row halo'd
slice packed as 2 channel blocks on 128 partitions. h/g stored with 2
replicate-padded columns per side (row stride 132) so dilated taps are
strided AP views.

v3.1 engine plan (per core):
- Head is chunked (512-col DMA + matmul + relu pipeline) with a short PE
  warm spin so HAM unthrottles early and stays warm; the box filter runs
  on the DVE inside PE/DMA shadows.
- k-loop runs at quarter granularity (8 out rows, FD=1024) with a 2-tap
  software pipeline: prod mul (DVE) -> bo reduce+broadcast (PE) -> nb copy
  (ACT) -> pk mul (DVE) -> facc accumulate (PE, identity matmul).
- cf chains (n_4/sum n^2, bc2 broadcast) are emitted per quarter right
  after their inputs exist so only the last quarter's chain is exposed.
- Residual and output are bf16 (tolerance is 2e-2).
"""
import sys
import contextlib
import numpy as np

sys.path.insert(0, '/opt/trn_rl_repo')

import concourse.bass as bass  # noqa: E402,F401
import concourse.bacc as bacc  # noqa: E402
import concourse.tile as tile  # noqa: E402
import concourse.mybir as mybir  # noqa: E402
from concourse.bass_utils import run_bass_kernel_spmd  # noqa: E402

F32 = mybir.dt.float32
BF16 = mybir.dt.bfloat16
AT = mybir.ActivationFunctionType
OP = mybir.AluOpType

N_CORES = 8
RIN = 36          # per-block input rows (with +-2 halo)
ROUT = 32         # per-block output rows
W = 128
WP = W + 4        # padded row stride for h/g
FIN = RIN * W     # 4608
FOUT = ROUT * W   # 4096
HF = 2048         # half (16 out rows)
QF = 1024         # quarter (8 out rows)
OFFS = [(di, dj) for di in (-2, 0, 2) for dj in (-2, 0, 2)]
KR = [(k - 4) % 9 for k in range(9)]   # tap k -> nst row pair index

EDT = BF16
N_WARM = 12


def _build():
    nc = bacc.Bacc("TRN2", target_bir_lowering=False, debug=False,
                   num_devices=N_CORES)
    xb_ext = nc.dram_tensor("xb", [128, FIN], EDT, kind="ExternalInput").ap()
    xr_ext = nc.dram_tensor("xr", [128, FOUT], EDT, kind="ExternalInput").ap()
    w2_ext = nc.dram_tensor("w2", [128, 128], EDT, kind="ExternalInput").ap()
    w3_ext = nc.dram_tensor("w3", [128, 128], EDT, kind="ExternalInput").ap()
    bo_ext = nc.dram_tensor("bo", [128, 128], EDT, kind="ExternalInput").ap()
    sb_ext = nc.dram_tensor("sb", [18, 2], EDT, kind="ExternalInput").ap()
    bc2_ext = nc.dram_tensor("bc2", [2, 128], EDT, kind="ExternalInput").ap()
    id_ext = nc.dram_tensor("idm", [128, 128], EDT, kind="ExternalInput").ap()
    y_ext = nc.dram_tensor("y", [128, FOUT], EDT, kind="ExternalOutput").ap()

    with tile.TileContext(nc) as tc, contextlib.ExitStack() as ctx:
        cpool = ctx.enter_context(tc.tile_pool(name="consts", bufs=1))
        big = ctx.enter_context(tc.tile_pool(name="big", bufs=1))
        ppool = ctx.enter_context(tc.tile_pool(name="prod", bufs=11))
        kpool = ctx.enter_context(tc.tile_pool(name="pk", bufs=3))
        nbpool = ctx.enter_context(tc.tile_pool(name="nbp", bufs=4))
        spool = ctx.enter_context(tc.tile_pool(name="small", bufs=4))

        # ---- input DMAs ----
        w2 = cpool.tile([128, 128], EDT)
        nc.sync.dma_start(w2[:], w2_ext[:])
        xbt = big.tile([128, FIN], EDT)
        for c in range(9):
            nc.sync.dma_start(xbt[:, c * 512:(c + 1) * 512],
                              xb_ext[:, c * 512:(c + 1) * 512])
        w3 = cpool.tile([128, 128], EDT)
        nc.sync.dma_start(w3[:], w3_ext[:])
        bo = cpool.tile([128, 128], EDT)
        nc.gpsimd.dma_start(bo[:], bo_ext[:])
        sbm = cpool.tile([18, 2], EDT)
        nc.gpsimd.dma_start(sbm[:], sb_ext[:])
        bc2 = cpool.tile([2, 128], EDT)
        nc.gpsimd.dma_start(bc2[:], bc2_ext[:])
        idm = cpool.tile([128, 128], EDT)
        nc.gpsimd.dma_start(idm[:], id_ext[:])
        xrt = big.tile([128, FOUT], EDT)
        for c in range(2):
            nc.sync.dma_start(xrt[:, c * HF:(c + 1) * HF],
                              xr_ext[:, c * HF:(c + 1) * HF])

        hf = big.tile([128, RIN * WP], EDT)
        gf = big.tile([128, RIN * WP], EDT)
        h3 = hf.rearrange("p (r w) -> p r w", w=WP)
        g3 = gf.rearrange("p (r w) -> p r w", w=WP)

        def tap(t3, di, dj, rows, r0):
            return t3[:, r0 + di:r0 + di + rows, 2 + dj:2 + dj + W]

        # ---- head: warm spin + h/g matmuls, chunked ----
        with tc.tile_pool(name="warm", bufs=1, space="PSUM") as wpool, \
                tc.tile_pool(name="psmm", bufs=4, space="PSUM") as psmm:
            warm_ps = wpool.tile([128, 128], F32)
            for i in range(N_WARM):
                nc.tensor.matmul(warm_ps[:], w2[:], w2[:, 0:128],
                                 start=(i == 0), stop=(i == N_WARM - 1),
                                 skip_group_check=True)

            for c in range(9):
                ps = psmm.tile([128, 512], F32)
                nc.tensor.matmul(ps[:], w2[:], xbt[:, c * 512:(c + 1) * 512],
                                 start=True, stop=True)
                nc.scalar.activation(
                    h3[:, 4 * c:4 * c + 4, 2:2 + W],
                    ps[:].rearrange("p (r w) -> p r w", w=W), AT.Relu)
            for c in range(9):
                ps = psmm.tile([128, 512], F32)
                nc.tensor.matmul(ps[:], w3[:], h3[:, 4 * c:4 * c + 4, 2:2 + W],
                                 start=True, stop=True)
                nc.scalar.copy(g3[:, 4 * c:4 * c + 4, 2:2 + W],
                               ps[:].rearrange("p (r w) -> p r w", w=W))

            # ---- box filter on DVE, chunked behind the relu pipeline ----
            def pads(t3, r0, rows):
                for dst, src in ((0, 2), (1, 2), (130, 129), (131, 129)):
                    nc.vector.tensor_copy(t3[:, r0:r0 + rows, dst:dst + 1],
                                          t3[:, r0:r0 + rows, src:src + 1])

            T = big.tile([128, FIN], EDT)
            T3 = T.rearrange("p (r w) -> p r w", w=W)
            av = big.tile([128, FOUT], EDT)
            av3 = av.rearrange("p (r w) -> p r w", w=W)

            def boxT(r0, rows):
                nc.vector.tensor_add(T3[:, r0:r0 + rows, :],
                                     tap(h3, -2, -2, rows, 2 + r0),
                                     tap(h3, -2, 0, rows, 2 + r0))
                nc.vector.tensor_add(T3[:, r0:r0 + rows, :],
                                     T3[:, r0:r0 + rows, :],
                                     tap(h3, -2, 2, rows, 2 + r0))

            def boxav(r0, rows):
                # av[r] = T[r] + T[r+2] + T[r+4]
                nc.vector.tensor_add(av3[:, r0:r0 + rows, :],
                                     T3[:, r0:r0 + rows, :],
                                     T3[:, r0 + 2:r0 + 2 + rows, :])
                nc.vector.tensor_add(av3[:, r0:r0 + rows, :],
                                     av3[:, r0:r0 + rows, :],
                                     T3[:, r0 + 4:r0 + 4 + rows, :])

            pads(h3, 0, 8)       # rows 0..7   (after relu chunk 1)
            boxT(0, 8)
            pads(h3, 8, 12)      # rows 8..19  (after relu chunk 4)
            boxT(8, 12)
            boxav(0, 16)         # av half 0
            pads(h3, 20, 16)
            pads(g3, 20, 16)

            # half-1 box runs on the (otherwise idle) GPSIMD engine
            def gT(r0, rows):
                nc.gpsimd.tensor_add(T3[:, r0:r0 + rows, :],
                                     tap(h3, -2, -2, rows, 2 + r0),
                                     tap(h3, -2, 0, rows, 2 + r0))
                nc.gpsimd.tensor_add(T3[:, r0:r0 + rows, :],
                                     T3[:, r0:r0 + rows, :],
                                     tap(h3, -2, 2, rows, 2 + r0))

            gT(20, 16)
            nc.gpsimd.tensor_add(av3[:, 16:32, :], T3[:, 16:32, :],
                                 T3[:, 18:34, :])
            nc.gpsimd.tensor_add(av3[:, 16:32, :], av3[:, 16:32, :],
                                 T3[:, 20:36, :])

            # consume warm psum (dep long satisfied by emission time)
            wsc = spool.tile([128, 128], EDT, tag="wsc")
            nc.scalar.copy(wsc[:], warm_ps[:])

        # ---- k-loop state ----
        nst = cpool.tile([18, FOUT], EDT)
        nsq = cpool.tile([18, FOUT], EDT)
        facc_sb = big.tile([128, FOUT], EDT)

        pnb = ctx.enter_context(tc.tile_pool(name="pnb", bufs=2, space="PSUM"))
        pfacc = ctx.enter_context(
            tc.tile_pool(name="pfacc", bufs=1, space="PSUM"))
        pcf = ctx.enter_context(tc.tile_pool(name="pcf", bufs=1, space="PSUM"))

        facc_ps = {}
        prods = {}      # k -> [128, HF] prod tile shared by a quarter pair

        def quarter(q, inject=None):
            """Software-pipelined tap loop for one 8-row quarter. Prod muls
            run at FD=2048 on even quarters and are reused by the odd one."""
            rq = 8 * q
            qs = slice(q * QF, (q + 1) * QF)
            nbs = {}

            def stage1(k):
                di, dj = OFFS[k]
                if q % 2 == 0:
                    prod = ppool.tile([128, HF], EDT, tag="pp",
                                      name=f"pp{q}_{k}")
                    p3 = prod.rearrange("p (r w) -> p r w", w=W)
                    nc.vector.tensor_mul(p3[:], av3[:, rq:rq + 16, :],
                                         tap(h3, di, dj, 16, 2 + rq))
                    prods[k] = prod
                prod = prods[k]
                po = (q % 2) * QF
                nps = pnb.tile([128, QF], F32, tag="nb", name=f"nps{q}_{k}")
                for cc in range(2):
                    nc.tensor.matmul(nps[:, cc * 512:(cc + 1) * 512], bo[:],
                                     prod[:, po + cc * 512:po + (cc + 1) * 512],
                                     start=True, stop=True)
                nb = nbpool.tile([128, QF], EDT, tag="nb", name=f"nb{q}_{k}")
                nc.scalar.copy(nb[:], nps[:])
                r = 2 * KR[k]
                nc.sync.dma_start(nst[r:r + 2, qs], nb[0:65:64, :])
                nbs[k] = nb

            gpk = {2, 4} if q >= 2 else set()
            deferred = []

            def idms(k, pk, stop):
                for cc in range(2):
                    nc.tensor.matmul(facc_ps[q][:, cc * 512:(cc + 1) * 512],
                                     idm[:], pk[:, cc * 512:(cc + 1) * 512],
                                     start=(k == 0), stop=(stop and cc == 1),
                                     skip_group_check=True)

            def stage2(k):
                di, dj = OFFS[k]
                nb3 = nbs.pop(k).rearrange("p (r w) -> p r w", w=W)
                pk = kpool.tile([128, QF], EDT, tag="pk", name=f"pk{q}_{k}")
                p3 = pk.rearrange("p (r w) -> p r w", w=W)
                eng = nc.gpsimd if k in gpk else nc.vector
                eng.tensor_mul(p3[:], nb3[:], tap(g3, di, dj, 8, 2 + rq))
                if k in gpk:
                    deferred.append((k, pk))
                else:
                    idms(k, pk, stop=(k == 8 and not gpk))

            cfbs = None
            for k in range(9):
                stage1(k)
                if k == 4 and inject is not None:
                    inject()
                if k >= 2:
                    stage2(k - 2)
            stage2(7)
            cfbs = cf1(q)
            stage2(8)
            for i, (k, pk) in enumerate(deferred):
                idms(k, pk, stop=(i == len(deferred) - 1))
            if not deferred:
                # re-mark accumulation end on the PSUM tile via a no-op stop:
                pass
            return cfbs

        def cf1(q):
            """nsq -> s2 -> recip -> cfr -> bc2 -> cfb for quarter q."""
            qs = slice(q * QF, (q + 1) * QF)
            nc.scalar.activation(nsq[:, qs], nst[:, qs], AT.Square)
            s2ps = pcf.tile([2, QF], F32, tag="cf", name=f"s2ps{q}")
            for cc in range(2):
                nc.tensor.matmul(
                    s2ps[:, cc * 512:(cc + 1) * 512], sbm[:],
                    nsq[:, q * QF + cc * 512:q * QF + cc * 512 + 512],
                    start=True, stop=True)
            rcp = spool.tile([2, QF], F32, tag="rcp", name=f"rcp{q}", bufs=2)
            nc.vector.reciprocal_approx_fast(rcp[:], s2ps[:])
            cfr = spool.tile([2, QF], EDT, tag="cfr", name=f"cfr{q}", bufs=2)
            nc.vector.tensor_mul(cfr[:], nst[0:2, qs], rcp[:])
            cfbs = []
            for cc in range(2):
                pst = pcf.tile([128, 512], F32, tag="cf", name=f"pst{q}_{cc}")
                nc.tensor.matmul(pst[:], bc2[:], cfr[:, cc * 512:(cc + 1) * 512],
                                 start=True, stop=True)
                cfb = spool.tile([128, 512], EDT, tag="cfb",
                                 name=f"cfb{q}_{cc}")
                nc.scalar.copy(cfb[:], pst[:])
                cfbs.append(cfb)
            return cfbs

        def res_q_copy(q):
            nc.scalar.copy(facc_sb[:, q * QF:(q + 1) * QF], facc_ps[q][:])

        def res_q_dve(q, cfbs):
            for cc in range(2):
                sl = slice(q * QF + cc * 512, q * QF + (cc + 1) * 512)
                res = spool.tile([128, 512], EDT, tag="res",
                                 name=f"res{q}_{cc}")
                nc.vector.tensor_mul(res[:], facc_sb[:, sl], cfbs[cc][:])
                yt = spool.tile([128, 512], EDT, tag="yt", name=f"yt{q}_{cc}")
                nc.vector.tensor_add(yt[:], res[:], xrt[:, sl])
                nc.sync.dma_start(y_ext[:, sl], yt[:])

        # ================= emission schedule =================
        facc_ps[0] = pfacc.tile([128, QF], F32, tag="fa", name="faccps0")

        pads(g3, 0, 20)        # g pads for q0/q1 pk taps
        cfb0 = quarter(0)
        res_q_copy(0)
        facc_ps[1] = pfacc.tile([128, QF], F32, tag="fa", name="faccps1")
        cfb1 = quarter(1, inject=lambda: res_q_dve(0, cfb0))
        res_q_copy(1)
        facc_ps[2] = pfacc.tile([128, QF], F32, tag="fa", name="faccps2")
        cfb2 = quarter(2, inject=lambda: res_q_dve(1, cfb1))
        res_q_copy(2)
        facc_ps[3] = pfacc.tile([128, QF], F32, tag="fa", name="faccps3")
        cfb3 = quarter(3, inject=lambda: res_q_dve(2, cfb2))
        for cc in range(2):
            sl = slice(3 * QF + cc * 512, 3 * QF + (cc + 1) * 512)
            res = spool.tile([128, 512], EDT, tag="res", name=f"res3_{cc}")
            nc.vector.tensor_mul(res[:], facc_ps[3][:, cc * 512:(cc + 1) * 512],
                                 cfb3[cc][:])
            yt = spool.tile([128, 512], EDT, tag="yt", name=f"yt3_{cc}")
            nc.vector.tensor_add(yt[:], res[:], xrt[:, sl])
            nc.sync.dma_start(y_ext[:, sl], yt[:])

    nc.compile()
    return nc


_NC_CACHE = [None]


def _get_nc():
    if _NC_CACHE[0] is None:
        _NC_CACHE[0] = _build()
    return _NC_CACHE[0]


def _host_prep(x):
    import ml_dtypes
    B, Cc, H, Ww = x.shape
    in_maps = []
    for core in range(N_CORES):
        b, half = core // 2, core % 2
        r0 = 64 * half
        gidx = np.clip(np.arange(r0 - 2, r0 + 66), 0, H - 1)
        xs = x[b][:, gidx, :]                     # (64, 68, 128)
        packed = np.ascontiguousarray(
            np.concatenate([xs[:, 0:36], xs[:, 32:68]], axis=0))
        xres = np.ascontiguousarray(packed[:, 2:34]).reshape(128, FOUT)
        in_maps.append({
            "xb": packed.reshape(128, FIN).astype(ml_dtypes.bfloat16),
            "xr": xres.astype(ml_dtypes.bfloat16),
        })
    return in_maps


def _const_maps(W_head, W_tail):
    import ml_dtypes

    def to_edt(a):
        return a.astype(ml_dtypes.bfloat16)

    w2 = np.zeros((128, 128), np.float32)
    w2[:64, :64] = W_head.T
    w2[64:, 64:] = W_head.T
    w3 = np.zeros((128, 128), np.float32)
    w3[:64, :64] = W_tail.T
    w3[64:, 64:] = W_tail.T
    bo = np.zeros((128, 128), np.float32)
    bo[:64, :64] = 1.0 / 9.0
    bo[64:, 64:] = 1.0 / 9.0
    sb = np.zeros((18, 2), np.float32)
    sb[0::2, 0] = 1.0
    sb[1::2, 1] = 1.0
    bc2 = np.zeros((2, 128), np.float32)
    bc2[0, :64] = 1.0
    bc2[1, 64:] = 1.0
    return {"w2": to_edt(w2), "w3": to_edt(w3), "bo": to_edt(bo),
            "sb": to_edt(sb), "bc2": to_edt(bc2),
            "idm": to_edt(np.eye(128, dtype=np.float32))}


def kernel(x, W_head, W_tail):
    x = np.asarray(x, np.float32)
    W_head = np.asarray(W_head, np.float32)
    W_tail = np.asarray(W_tail, np.float32)
    nc = _get_nc()
    consts = _const_maps(W_head, W_tail)
    in_maps = [{**m, **consts} for m in _host_prep(x)]
    res = run_bass_kernel_spmd(nc, in_maps, list(range(N_CORES)))
    out = np.empty_like(x)
    for core in range(N_CORES):
        b, half = core // 2, core % 2
        r0 = 64 * half
        y = res.results[core]["y"].astype(np.float32).reshape(128, ROUT, W)
        out[b, :, r0:r0 + 32, :] = y[:64]
        out[b, :, r0 + 32:r0 + 64, :] = y[64:]
    return out


# revision 16
# speedup vs baseline: 1.1642x; 1.1642x over previous
"""Trainium2 Bass kernel for nn_LowRankLayer_dilation (B=4, C=64, H=W=128).

Math (rank-3 NMF collapses exactly; eps negligible):
    h   = relu(W_head @ x)            per-pixel channel matmul
    g   = W_tail @ h
    a   = box9(h)                     3x3 dilation-2 box sum, edge-clamped
    n_k = sum_c (a/9)_c * h_c(p+d_k)  9 taps, d in {-2,0,2}^2
    out = x + (n_4 / sum_j n_j^2) * sum_k n_k * g(p+d_k)

Sharding: pure data parallel, 8 cores = (batch, H-half); 68-row halo'd
slice packed as 2 channel blocks on 128 partitions. h/g stored with 2
replicate-padded columns per side (row stride 132) so dilated taps are
strided AP views.

v3.1 engine plan (per core):
- Head is chunked (512-col DMA + matmul + relu pipeline) with a short PE
  warm spin so HAM unthrottles early and stays warm; the box filter runs
  on the DVE inside PE/DMA shadows.
- k-loop runs at quarter granularity (8 out rows, FD=1024) with a 2-tap
  software pipeline: prod mul (DVE) -> bo reduce+broadcast (PE) -> nb copy
  (ACT) -> pk mul (DVE) -> facc accumulate (PE, identity matmul).
- cf chains (n_4/sum n^2, bc2 broadcast) are emitted per quarter right
  after their inputs exist so only the last quarter's chain is exposed.
- Residual and output are bf16 (tolerance is 2e-2).
"""
import sys
import contextlib
import numpy as np

sys.path.insert(0, '/opt/trn_rl_repo')

import concourse.bass as bass  # noqa: E402,F401
import concourse.bacc as bacc  # noqa: E402
import concourse.tile as tile  # noqa: E402
import concourse.mybir as mybir  # noqa: E402
from concourse.bass_utils import run_bass_kernel_spmd  # noqa: E402

F32 = mybir.dt.float32
BF16 = mybir.dt.bfloat16
AT = mybir.ActivationFunctionType
OP = mybir.AluOpType

N_CORES = 8
RIN = 36          # per-block input rows (with +-2 halo)
ROUT = 32         # per-block output rows
W = 128
WP = W + 4        # padded row stride for h/g
FIN = RIN * W     # 4608
FOUT = ROUT * W   # 4096
HF = 2048         # half (16 out rows)
QF = 1024         # quarter (8 out rows)
OFFS = [(di, dj) for di in (-2, 0, 2) for dj in (-2, 0, 2)]
KR = [(k - 4) % 9 for k in range(9)]   # tap k -> nst row pair index

EDT = BF16
N_WARM = 12


def _build():
    nc = bacc.Bacc("TRN2", target_bir_lowering=False, debug=False,
                   num_devices=N_CORES)
    xb_ext = nc.dram_tensor("xb", [128, FIN], EDT, kind="ExternalInput").ap()
    xr_ext = nc.dram_tensor("xr", [128, FOUT], EDT, kind="ExternalInput").ap()
    w2_ext = nc.dram_tensor("w2", [128, 128], EDT, kind="ExternalInput").ap()
    w3_ext = nc.dram_tensor("w3", [128, 128], EDT, kind="ExternalInput").ap()
    bo_ext = nc.dram_tensor("bo", [128, 128], EDT, kind="ExternalInput").ap()
    sb_ext = nc.dram_tensor("sb", [18, 2], EDT, kind="ExternalInput").ap()
    bc2_ext = nc.dram_tensor("bc2", [2, 128], EDT, kind="ExternalInput").ap()
    id_ext = nc.dram_tensor("idm", [128, 128], EDT, kind="ExternalInput").ap()
    y_ext = nc.dram_tensor("y", [128, FOUT], EDT, kind="ExternalOutput").ap()

    with tile.TileContext(nc) as tc, contextlib.ExitStack() as ctx:
        cpool = ctx.enter_context(tc.tile_pool(name="consts", bufs=1))
        big = ctx.enter_context(tc.tile_pool(name="big", bufs=1))
        ppool = ctx.enter_context(tc.tile_pool(name="prod", bufs=11))
        kpool = ctx.enter_context(tc.tile_pool(name="pk", bufs=3))
        nbpool = ctx.enter_context(tc.tile_pool(name="nbp", bufs=4))
        spool = ctx.enter_context(tc.tile_pool(name="small", bufs=4))

        # ---- input DMAs ----
        w2 = cpool.tile([128, 128], EDT)
        nc.sync.dma_start(w2[:], w2_ext[:])
        xbt = big.tile([128, FIN], EDT)
        for c in range(9):
            nc.sync.dma_start(xbt[:, c * 512:(c + 1) * 512],
                              xb_ext[:, c * 512:(c + 1) * 512])
        w3 = cpool.tile([128, 128], EDT)
        nc.sync.dma_start(w3[:], w3_ext[:])
        bo = cpool.tile([128, 128], EDT)
        nc.gpsimd.dma_start(bo[:], bo_ext[:])
        sbm = cpool.tile([18, 2], EDT)
        nc.gpsimd.dma_start(sbm[:], sb_ext[:])
        bc2 = cpool.tile([2, 128], EDT)
        nc.gpsimd.dma_start(bc2[:], bc2_ext[:])
        idm = cpool.tile([128, 128], EDT)
        nc.gpsimd.dma_start(idm[:], id_ext[:])
        xrt = big.tile([128, FOUT], EDT)
        for c in range(2):
            nc.sync.dma_start(xrt[:, c * HF:(c + 1) * HF],
                              xr_ext[:, c * HF:(c + 1) * HF])

        hf = big.tile([128, RIN * WP], EDT)
        gf = big.tile([128, RIN * WP], EDT)
        h3 = hf.rearrange("p (r w) -> p r w", w=WP)
        g3 = gf.rearrange("p (r w) -> p r w", w=WP)

        def tap(t3, di, dj, rows, r0):
            return t3[:, r0 + di:r0 + di + rows, 2 + dj:2 + dj + W]

        # ---- head: warm spin + h/g matmuls, chunked ----
        with tc.tile_pool(name="warm", bufs=1, space="PSUM") as wpool, \
                tc.tile_pool(name="psmm", bufs=4, space="PSUM") as psmm:
            warm_ps = wpool.tile([128, 128], F32)
            for i in range(N_WARM):
                nc.tensor.matmul(warm_ps[:], w2[:], w2[:, 0:128],
                                 start=(i == 0), stop=(i == N_WARM - 1),
                                 skip_group_check=True)

            for c in range(9):
                ps = psmm.tile([128, 512], F32)
                nc.tensor.matmul(ps[:], w2[:], xbt[:, c * 512:(c + 1) * 512],
                                 start=True, stop=True)
                nc.scalar.activation(
                    h3[:, 4 * c:4 * c + 4, 2:2 + W],
                    ps[:].rearrange("p (r w) -> p r w", w=W), AT.Relu)
            for c in range(9):
                ps = psmm.tile([128, 512], F32)
                nc.tensor.matmul(ps[:], w3[:], h3[:, 4 * c:4 * c + 4, 2:2 + W],
                                 start=True, stop=True)
                nc.scalar.copy(g3[:, 4 * c:4 * c + 4, 2:2 + W],
                               ps[:].rearrange("p (r w) -> p r w", w=W))

            # ---- box filter on DVE, chunked behind the relu pipeline ----
            def pads(t3, r0, rows):
                for dst, src in ((0, 2), (1, 2), (130, 129), (131, 129)):
                    nc.vector.tensor_copy(t3[:, r0:r0 + rows, dst:dst + 1],
                                          t3[:, r0:r0 + rows, src:src + 1])

            T = big.tile([128, FIN], EDT)
            T3 = T.rearrange("p (r w) -> p r w", w=W)
            av = big.tile([128, FOUT], EDT)
            av3 = av.rearrange("p (r w) -> p r w", w=W)

            def boxT(r0, rows):
                nc.vector.tensor_add(T3[:, r0:r0 + rows, :],
                                     tap(h3, -2, -2, rows, 2 + r0),
                                     tap(h3, -2, 0, rows, 2 + r0))
                nc.vector.tensor_add(T3[:, r0:r0 + rows, :],
                                     T3[:, r0:r0 + rows, :],
                                     tap(h3, -2, 2, rows, 2 + r0))

            def boxav(r0, rows):
                # av[r] = T[r] + T[r+2] + T[r+4]
                nc.vector.tensor_add(av3[:, r0:r0 + rows, :],
                                     T3[:, r0:r0 + rows, :],
                                     T3[:, r0 + 2:r0 + 2 + rows, :])
                nc.vector.tensor_add(av3[:, r0:r0 + rows, :],
                                     av3[:, r0:r0 + rows, :],
                                     T3[:, r0 + 4:r0 + 4 + rows, :])

            pads(h3, 0, 8)       # rows 0..7   (after relu chunk 1)
            boxT(0, 8)
            pads(h3, 8, 12)      # rows 8..19  (after relu chunk 4)
            boxT(8, 12)
            boxav(0, 16)         # av half 0
            pads(h3, 20, 16)
            pads(g3, 20, 16)

            # consume warm psum (dep long satisfied by emission time)
            wsc = spool.tile([128, 128], EDT, tag="wsc")
            nc.scalar.copy(wsc[:], warm_ps[:])

        # ---- k-loop state ----
        nst = cpool.tile([18, FOUT], EDT)
        nsq = cpool.tile([18, FOUT], EDT)
        facc_sb = big.tile([128, FOUT], EDT)

        pnb = ctx.enter_context(tc.tile_pool(name="pnb", bufs=2, space="PSUM"))
        pfacc = ctx.enter_context(
            tc.tile_pool(name="pfacc", bufs=1, space="PSUM"))
        pcf = ctx.enter_context(tc.tile_pool(name="pcf", bufs=1, space="PSUM"))

        facc_ps = {}
        prods = {}      # k -> [128, HF] prod tile shared by a quarter pair

        def quarter(q, inject=None):
            """Software-pipelined tap loop for one 8-row quarter. Prod muls
            run at FD=2048 on even quarters and are reused by the odd one."""
            rq = 8 * q
            qs = slice(q * QF, (q + 1) * QF)
            nbs = {}

            def stage1(k):
                di, dj = OFFS[k]
                if q % 2 == 0:
                    prod = ppool.tile([128, HF], EDT, tag="pp",
                                      name=f"pp{q}_{k}")
                    p3 = prod.rearrange("p (r w) -> p r w", w=W)
                    nc.vector.tensor_mul(p3[:], av3[:, rq:rq + 16, :],
                                         tap(h3, di, dj, 16, 2 + rq))
                    prods[k] = prod
                prod = prods[k]
                po = (q % 2) * QF
                nps = pnb.tile([128, QF], F32, tag="nb", name=f"nps{q}_{k}")
                for cc in range(2):
                    nc.tensor.matmul(nps[:, cc * 512:(cc + 1) * 512], bo[:],
                                     prod[:, po + cc * 512:po + (cc + 1) * 512],
                                     start=True, stop=True)
                nb = nbpool.tile([128, QF], EDT, tag="nb", name=f"nb{q}_{k}")
                nc.scalar.copy(nb[:], nps[:])
                r = 2 * KR[k]
                nc.sync.dma_start(nst[r:r + 2, qs], nb[0:65:64, :])
                nbs[k] = nb

            gpk = set()
            deferred = []

            def idms(k, pk, stop):
                for cc in range(2):
                    nc.tensor.matmul(facc_ps[q][:, cc * 512:(cc + 1) * 512],
                                     idm[:], pk[:, cc * 512:(cc + 1) * 512],
                                     start=(k == 0), stop=(stop and cc == 1),
                                     skip_group_check=True)

            def stage2(k):
                di, dj = OFFS[k]
                nb3 = nbs.pop(k).rearrange("p (r w) -> p r w", w=W)
                pk = kpool.tile([128, QF], EDT, tag="pk", name=f"pk{q}_{k}")
                p3 = pk.rearrange("p (r w) -> p r w", w=W)
                eng = nc.gpsimd if k in gpk else nc.vector
                eng.tensor_mul(p3[:], nb3[:], tap(g3, di, dj, 8, 2 + rq))
                if k in gpk:
                    deferred.append((k, pk))
                else:
                    idms(k, pk, stop=(k == 8 and not gpk))

            cfbs = None
            for k in range(9):
                stage1(k)
                if k == 4 and inject is not None:
                    inject()
                if k >= 2:
                    stage2(k - 2)
            stage2(7)
            cfbs = cf1(q)
            stage2(8)
            for i, (k, pk) in enumerate(deferred):
                idms(k, pk, stop=(i == len(deferred) - 1))
            if not deferred:
                # re-mark accumulation end on the PSUM tile via a no-op stop:
                pass
            return cfbs

        def cf1(q):
            """nsq -> s2 -> recip -> cfr -> bc2 -> cfb for quarter q."""
            qs = slice(q * QF, (q + 1) * QF)
            nc.scalar.activation(nsq[:, qs], nst[:, qs], AT.Square)
            s2ps = pcf.tile([2, QF], F32, tag="cf", name=f"s2ps{q}")
            for cc in range(2):
                nc.tensor.matmul(
                    s2ps[:, cc * 512:(cc + 1) * 512], sbm[:],
                    nsq[:, q * QF + cc * 512:q * QF + cc * 512 + 512],
                    start=True, stop=True)
            rcp = spool.tile([2, QF], F32, tag="rcp", name=f"rcp{q}", bufs=2)
            nc.vector.reciprocal_approx_fast(rcp[:], s2ps[:])
            cfr = spool.tile([2, QF], EDT, tag="cfr", name=f"cfr{q}", bufs=2)
            nc.vector.tensor_mul(cfr[:], nst[0:2, qs], rcp[:])
            cfbs = []
            for cc in range(2):
                pst = pcf.tile([128, 512], F32, tag="cf", name=f"pst{q}_{cc}")
                nc.tensor.matmul(pst[:], bc2[:], cfr[:, cc * 512:(cc + 1) * 512],
                                 start=True, stop=True)
                cfb = spool.tile([128, 512], EDT, tag="cfb",
                                 name=f"cfb{q}_{cc}")
                nc.scalar.copy(cfb[:], pst[:])
                cfbs.append(cfb)
            return cfbs

        def res_q_copy(q):
            nc.scalar.copy(facc_sb[:, q * QF:(q + 1) * QF], facc_ps[q][:])

        def res_q_dve(q, cfbs):
            for cc in range(2):
                sl = slice(q * QF + cc * 512, q * QF + (cc + 1) * 512)
                res = spool.tile([128, 512], EDT, tag="res",
                                 name=f"res{q}_{cc}")
                nc.vector.tensor_mul(res[:], facc_sb[:, sl], cfbs[cc][:])
                yt = spool.tile([128, 512], EDT, tag="yt", name=f"yt{q}_{cc}")
                nc.vector.tensor_add(yt[:], res[:], xrt[:, sl])
                nc.sync.dma_start(y_ext[:, sl], yt[:])

        # ================= emission schedule =================
        facc_ps[0] = pfacc.tile([128, QF], F32, tag="fa", name="faccps0")

        def box_h1():
            boxT(20, 16)
            boxav(16, 16)      # av half 1

        pads(g3, 0, 20)        # g pads for q0/q1 pk taps
        cfb0 = quarter(0, inject=box_h1)
        res_q_copy(0)
        facc_ps[1] = pfacc.tile([128, QF], F32, tag="fa", name="faccps1")
        cfb1 = quarter(1, inject=lambda: res_q_dve(0, cfb0))
        res_q_copy(1)
        facc_ps[2] = pfacc.tile([128, QF], F32, tag="fa", name="faccps2")
        cfb2 = quarter(2, inject=lambda: res_q_dve(1, cfb1))
        res_q_copy(2)
        facc_ps[3] = pfacc.tile([128, QF], F32, tag="fa", name="faccps3")
        cfb3 = quarter(3, inject=lambda: res_q_dve(2, cfb2))
        for cc in range(2):
            sl = slice(3 * QF + cc * 512, 3 * QF + (cc + 1) * 512)
            res = spool.tile([128, 512], EDT, tag="res", name=f"res3_{cc}")
            nc.vector.tensor_mul(res[:], facc_ps[3][:, cc * 512:(cc + 1) * 512],
                                 cfb3[cc][:])
            yt = spool.tile([128, 512], EDT, tag="yt", name=f"yt3_{cc}")
            nc.vector.tensor_add(yt[:], res[:], xrt[:, sl])
            nc.sync.dma_start(y_ext[:, sl], yt[:])

    nc.compile()
    return nc


_NC_CACHE = [None]


def _get_nc():
    if _NC_CACHE[0] is None:
        _NC_CACHE[0] = _build()
    return _NC_CACHE[0]


def _host_prep(x):
    import ml_dtypes
    B, Cc, H, Ww = x.shape
    in_maps = []
    for core in range(N_CORES):
        b, half = core // 2, core % 2
        r0 = 64 * half
        gidx = np.clip(np.arange(r0 - 2, r0 + 66), 0, H - 1)
        xs = x[b][:, gidx, :]                     # (64, 68, 128)
        packed = np.ascontiguousarray(
            np.concatenate([xs[:, 0:36], xs[:, 32:68]], axis=0))
        xres = np.ascontiguousarray(packed[:, 2:34]).reshape(128, FOUT)
        in_maps.append({
            "xb": packed.reshape(128, FIN).astype(ml_dtypes.bfloat16),
            "xr": xres.astype(ml_dtypes.bfloat16),
        })
    return in_maps


def _const_maps(W_head, W_tail):
    import ml_dtypes

    def to_edt(a):
        return a.astype(ml_dtypes.bfloat16)

    w2 = np.zeros((128, 128), np.float32)
    w2[:64, :64] = W_head.T
    w2[64:, 64:] = W_head.T
    w3 = np.zeros((128, 128), np.float32)
    w3[:64, :64] = W_tail.T
    w3[64:, 64:] = W_tail.T
    bo = np.zeros((128, 128), np.float32)
    bo[:64, :64] = 1.0 / 9.0
    bo[64:, 64:] = 1.0 / 9.0
    sb = np.zeros((18, 2), np.float32)
    sb[0::2, 0] = 1.0
    sb[1::2, 1] = 1.0
    bc2 = np.zeros((2, 128), np.float32)
    bc2[0, :64] = 1.0
    bc2[1, 64:] = 1.0
    return {"w2": to_edt(w2), "w3": to_edt(w3), "bo": to_edt(bo),
            "sb": to_edt(sb), "bc2": to_edt(bc2),
            "idm": to_edt(np.eye(128, dtype=np.float32))}


def kernel(x, W_head, W_tail):
    x = np.asarray(x, np.float32)
    W_head = np.asarray(W_head, np.float32)
    W_tail = np.asarray(W_tail, np.float32)
    nc = _get_nc()
    consts = _const_maps(W_head, W_tail)
    in_maps = [{**m, **consts} for m in _host_prep(x)]
    res = run_bass_kernel_spmd(nc, in_maps, list(range(N_CORES)))
    out = np.empty_like(x)
    for core in range(N_CORES):
        b, half = core // 2, core % 2
        r0 = 64 * half
        y = res.results[core]["y"].astype(np.float32).reshape(128, ROUT, W)
        out[b, :, r0:r0 + 32, :] = y[:64]
        out[b, :, r0 + 32:r0 + 64, :] = y[64:]
    return out
